# revision 1
# baseline (speedup 1.0000x reference)
"""Trainium2 Bass kernel for nn_ContrastMemLoss (SupCon distillation loss).

Self-contained: hardcodes all shapes. Distributes over 8 NeuronCores:
- data-parallel over the batch dim (1 image/core) for the BN statistics
  streaming phase (the only part that touches the 2x256MiB inputs),
- 8KB AllReduce of the BN moment vectors,
- anchor embeddings computed per-core, MxM similarity matrix row-sharded,
- per-core partial loss sums reduced on host.
"""
import sys

if "/opt/trn_rl_repo" not in sys.path:
    sys.path.insert(0, "/opt/trn_rl_repo")

import numpy as np
import ml_dtypes

import concourse.bacc as bacc
import concourse.mybir as mybir
import concourse.tile as tile
import concourse.bass_utils as bass_utils
from concourse.bass import AP  # noqa: F401

F32 = mybir.dt.float32
BF16 = mybir.dt.bfloat16

TEMP = 0.07
BASE_TEMP = 0.07
LOSS_WEIGHT = 0.5
BN_EPS = 1e-5

NCORES = 8


class Dims:
    def __init__(self, C=512, HW=16384, D=256, A=152, V=26, MP=4096, PT=2048):
        self.C = C          # conv channels
        self.HW = HW        # pixels per image
        self.D = D          # feature dim
        self.A = A          # anchors
        self.V = V          # views
        self.M = A * V      # anchor matrix size (unpadded)
        self.MP = MP        # padded (multiple of 512*NCORES/...)
        self.PT = PT        # pixel tile (free dim) for streaming phase
        self.KB = C // 128  # cin blocks
        self.CB = C // 128  # cout blocks (conv1)
        self.ZB = D // 128  # cout blocks (conv2)
        self.NT = HW // PT  # pixel tiles per image
        self.NCHUNK = PT // 512       # 512-pixel matmul chunks per tile
        self.MC = MP // 512           # anchor column chunks
        self.RPC = MP // NCORES       # rows per core
        self.RB = self.RPC // 128     # row blocks per core
        self.NPIX = NCORES * HW       # total pixels (BN denominator)


def build_kernel(dims: Dims, phases: int = 4):
    d = dims
    nc = bacc.Bacc("TRN2", target_bir_lowering=False, debug=False, num_devices=NCORES)

    # ---- per-core DRAM I/O ----
    ximg = nc.dram_tensor("ximg", [2, d.KB, 128, d.HW], BF16, kind="ExternalInput")
    xst = nc.dram_tensor("xst", [2, d.KB, 128, d.MP], BF16, kind="ExternalInput")
    xso = nc.dram_tensor("xso", [2, d.KB, 128, d.RPC], BF16, kind="ExternalInput")
    w1t = nc.dram_tensor("w1t", [2, d.KB, 128, d.C], BF16, kind="ExternalInput")
    w2t = nc.dram_tensor("w2t", [2, d.KB, 128, d.D], BF16, kind="ExternalInput")
    # per-stage channel vectors in [128, CB] layout (c = f*128 + p): gamma, beta, b1
    bnc = nc.dram_tensor("bnc", [2, 3, 128, d.CB], F32, kind="ExternalInput")
    b2c = nc.dram_tensor("b2c", [2, 128, d.ZB], F32, kind="ExternalInput")
    maskp = nc.dram_tensor("maskp", [d.RB, 128, d.MP], BF16, kind="ExternalInput")
    maskn = nc.dram_tensor("maskn", [d.RB, 128, d.MP], BF16, kind="ExternalInput")
    rowco = nc.dram_tensor("rowco", [128, d.RB], F32, kind="ExternalInput")
    pout = nc.dram_tensor("pout", [128, 2 * d.RB], F32, kind="ExternalOutput")

    inv_npix = 1.0 / float(d.NPIX)

    with tile.TileContext(nc) as tc:
        with (
            tc.tile_pool(name="wpool", bufs=1) as wpool,
            tc.tile_pool(name="zpool", bufs=1) as zpool,
            tc.tile_pool(name="cpool", bufs=1) as cpool,
            tc.tile_pool(name="dram", bufs=1, space="DRAM") as dram,
        ):
            # resident weights
            w1sb = [[wpool.tile([128, d.C], BF16, tag=f"w1_{st}_{kb}", name=f"w1_{st}_{kb}") for kb in range(d.KB)] for st in range(2)]
            w2sb = [[wpool.tile([128, d.D], BF16, tag=f"w2_{st}_{kb}", name=f"w2_{st}_{kb}") for kb in range(d.KB)] for st in range(2)]
            for st in range(2):
                for kb in range(d.KB):
                    nc.sync.dma_start(w1sb[st][kb][:], w1t[st, kb])
                    nc.sync.dma_start(w2sb[st][kb][:], w2t[st, kb])
            # resident anchor embeddings (bf16, channel-major)
            ZT = [[zpool.tile([128, d.MP], BF16, tag=f"zt{st}{zb}", name=f"zt{st}{zb}") for zb in range(d.ZB)] for st in range(2)]
            ZO = [[zpool.tile([128, d.RPC], BF16, tag=f"zo{st}{zb}", name=f"zo{st}{zb}") for zb in range(d.ZB)] for st in range(2)]
            # constants
            gsb = [cpool.tile([128, d.CB], F32, tag=f"g{st}", name=f"g{st}") for st in range(2)]
            bsb = [cpool.tile([128, d.CB], F32, tag=f"b{st}", name=f"b{st}") for st in range(2)]
            b1sb = [cpool.tile([128, d.CB], F32, tag=f"b1{st}", name=f"b1{st}") for st in range(2)]
            b2sb = [cpool.tile([128, d.ZB], F32, tag=f"b2{st}", name=f"b2{st}") for st in range(2)]
            for st in range(2):
                nc.sync.dma_start(gsb[st][:], bnc[st, 0])
                nc.sync.dma_start(bsb[st][:], bnc[st, 1])
                nc.sync.dma_start(b1sb[st][:], bnc[st, 2])
                nc.sync.dma_start(b2sb[st][:], b2c[st])
            rcsb = cpool.tile([128, d.RB], F32, tag="rc")
            nc.sync.dma_start(rcsb[:], rowco[:])
            ones_col = cpool.tile([128, 1], BF16, tag="ones_col")
            nc.vector.memset(ones_col[:], 1.0)
            ones_row = cpool.tile([1, 128], BF16, tag="ones_row")
            nc.vector.memset(ones_row[:], 1.0)
            # stats accumulators
            xsum_acc = cpool.tile([128, 2 * d.KB * d.NT], F32, tag="xsum_acc")
            y2_acc = cpool.tile([128, 2 * d.CB * d.NT * d.NCHUNK], F32, tag="y2_acc")
            stat_sb = cpool.tile([128, 16], F32, tag="stat")
            stat2_sb = cpool.tile([128, 16], F32, tag="stat2")
            scale_sb = [cpool.tile([128, d.CB], F32, tag=f"sc{st}", name=f"sc{st}") for st in range(2)]
            shift_sb = [cpool.tile([128, d.CB], F32, tag=f"sh{st}", name=f"sh{st}") for st in range(2)]
            out_sb = cpool.tile([128, 2 * d.RB], F32, tag="out")

            # ================= Phase 1: streaming BN moments =================
            with (
                tc.tile_pool(name="xstream", bufs=8) as xpool,
                tc.tile_pool(name="sq", bufs=4) as sqpool,
                tc.tile_pool(name="psum1", bufs=6, space="PSUM") as pp1,
            ):
                for st in range(2):
                    for t in range(d.NT):
                        xt = []
                        for kb in range(d.KB):
                            x = xpool.tile([128, d.PT], BF16, tag="x")
                            nc.sync.dma_start(x[:], ximg[st, kb, :, t * d.PT:(t + 1) * d.PT])
                            xt.append(x)
                            nc.vector.tensor_reduce(
                                xsum_acc[:, (st * d.KB + kb) * d.NT + t : (st * d.KB + kb) * d.NT + t + 1],
                                x[:], axis=mybir.AxisListType.X, op=mybir.AluOpType.add)
                        for ch in range(d.NCHUNK):
                            for cb in range(d.CB):
                                ps = pp1.tile([128, 512], F32)
                                for kb in range(d.KB):
                                    nc.tensor.matmul(
                                        ps[:],
                                        w1sb[st][kb][:, cb * 128:(cb + 1) * 128],
                                        xt[kb][:, ch * 512:(ch + 1) * 512],
                                        start=(kb == 0), stop=(kb == d.KB - 1))
                                sq = sqpool.tile([128, 512], BF16, tag="sq")
                                col = (st * d.CB + cb) * d.NT * d.NCHUNK + t * d.NCHUNK + ch
                                nc.scalar.activation(
                                    sq[:], ps[:], mybir.ActivationFunctionType.Square,
                                    accum_out=y2_acc[:, col:col + 1])

                # ---- finalize per-core moments + matvec mu ----
                xsum_bf = cpool.tile([128, 2 * d.KB], BF16, tag="xsum_bf")
                for st in range(2):
                    for kb in range(d.KB):
                        s = (st * d.KB + kb) * d.NT
                        col = st * d.KB + kb
                        with nc.allow_low_precision(reason="bf16 Sx for tiny mean matvec; error ~1e-6 of h scale"):
                            nc.vector.tensor_reduce(
                                xsum_bf[:, col:col + 1],
                                xsum_acc[:, s:s + d.NT], axis=mybir.AxisListType.X,
                                op=mybir.AluOpType.add)
                for st in range(2):
                    for cb in range(d.CB):
                        ps = pp1.tile([128, 1], F32, tag="mv", bufs=2)
                        for kb in range(d.KB):
                            nc.tensor.matmul(
                                ps[:],
                                w1sb[st][kb][:, cb * 128:(cb + 1) * 128],
                                xsum_bf[:, st * d.KB + kb: st * d.KB + kb + 1],
                                start=(kb == 0), stop=(kb == d.KB - 1))
                        # stat rows: r = 2*st -> mu, r = 2*st+1 -> y2 ; col = r*4 + cb
                        nc.vector.tensor_copy(stat_sb[:, (2 * st) * 4 + cb:(2 * st) * 4 + cb + 1], ps[:])
                        s = (st * d.CB + cb) * d.NT * d.NCHUNK
                        nc.vector.tensor_reduce(
                            stat_sb[:, (2 * st + 1) * 4 + cb:(2 * st + 1) * 4 + cb + 1],
                            y2_acc[:, s:s + d.NT * d.NCHUNK], axis=mybir.AxisListType.X,
                            op=mybir.AluOpType.add)

            # ================= Phase 2: AllReduce + BN params =================
            ar_in = dram.tile([128, 16], F32)
            ar_out = dram.tile([128, 16], F32)
            nc.sync.dma_start(ar_in[:], stat_sb[:])
            nc.gpsimd.collective_compute(
                "AllReduce", mybir.AluOpType.add,
                replica_groups=[list(range(NCORES))],
                ins=[ar_in.opt()], outs=[ar_out.opt()])
            nc.sync.dma_start(stat2_sb[:], ar_out[:])

            tmp_a = cpool.tile([128, d.CB], F32, tag="tmp_a")
            tmp_b = cpool.tile([128, d.CB], F32, tag="tmp_b")
            for st in range(2):
                mu = stat2_sb[:, (2 * st) * 4:(2 * st) * 4 + d.CB]
                y2 = stat2_sb[:, (2 * st + 1) * 4:(2 * st + 1) * 4 + d.CB]
                # mean = mu/NPIX ; var = y2/NPIX - mean^2
                nc.vector.tensor_scalar_mul(mu, mu, inv_npix)
                nc.vector.tensor_scalar_mul(y2, y2, inv_npix)
                nc.vector.tensor_mul(tmp_a[:], mu, mu)
                nc.vector.tensor_sub(tmp_a[:], y2, tmp_a[:])
                # inv_std = exp(-0.5*ln(var+eps))
                nc.vector.tensor_scalar_add(tmp_a[:], tmp_a[:], BN_EPS)
                nc.scalar.activation(tmp_b[:], tmp_a[:], mybir.ActivationFunctionType.Ln)
                nc.scalar.activation(tmp_a[:], tmp_b[:], mybir.ActivationFunctionType.Exp, scale=-0.5)
                nc.vector.tensor_mul(scale_sb[st][:], gsb[st][:], tmp_a[:])
                # shift = beta - (mean + b1) * scale
                nc.vector.tensor_add(tmp_a[:], mu, b1sb[st][:])
                nc.vector.tensor_mul(tmp_a[:], tmp_a[:], scale_sb[st][:])
                nc.vector.tensor_sub(shift_sb[st][:], bsb[st][:], tmp_a[:])

            # ================= Phase 3: anchor embeddings =================
            if phases < 3:
                nc.sync.dma_start(pout[:], stat2_sb[:, 0:8])
            with (
                tc.tile_pool(name="xa", bufs=8) as xapool,
                tc.tile_pool(name="hsb", bufs=8) as hpool,
                tc.tile_pool(name="zsb", bufs=4) as zspool,
                tc.tile_pool(name="nrm", bufs=2) as npool,
                tc.tile_pool(name="ph", bufs=4, space="PSUM") as pph,
                tc.tile_pool(name="pz", bufs=2, space="PSUM") as ppz,
                tc.tile_pool(name="pn", bufs=1, space="PSUM") as ppn,
                tc.tile_pool(name="pb", bufs=1, space="PSUM") as ppb,
            ):
                n_own = d.RPC // 512  # own-column chunks (RPC multiple of 512)
                for st in range(2 if phases >= 3 else 0):
                    for mc in range(d.MC + n_own):
                        own = mc >= d.MC
                        mcl = mc - d.MC if own else mc
                        src = xso if own else xst
                        width = 512
                        xa = []
                        for kb in range(d.KB):
                            x = xapool.tile([128, width], BF16, tag="xa")
                            nc.sync.dma_start(x[:], src[st, kb, :, mcl * 512:(mcl + 1) * 512])
                            xa.append(x)
                        hs = []
                        for cb in range(d.CB):
                            ph = pph.tile([128, width], F32)
                            for kb in range(d.KB):
                                nc.tensor.matmul(
                                    ph[:],
                                    w1sb[st][kb][:, cb * 128:(cb + 1) * 128],
                                    xa[kb][:],
                                    start=(kb == 0), stop=(kb == d.KB - 1))
                            h = hpool.tile([128, width], BF16, tag="h")
                            nc.scalar.activation(
                                h[:], ph[:], mybir.ActivationFunctionType.Relu,
                                bias=shift_sb[st][:, cb:cb + 1], scale=scale_sb[st][:, cb:cb + 1])
                            hs.append(h)
                        zs = []
                        zqs = []
                        pn = ppn.tile([1, width], F32, tag="pn")
                        for zb in range(d.ZB):
                            pz = ppz.tile([128, width], F32)
                            for cb in range(d.CB):
                                nc.tensor.matmul(
                                    pz[:],
                                    w2sb[st][cb][:, zb * 128:(zb + 1) * 128],
                                    hs[cb][:],
                                    start=(cb == 0), stop=(cb == d.CB - 1))
                            z = zspool.tile([128, width], F32, tag="z")
                            nc.vector.tensor_scalar_add(z[:], pz[:], b2sb[st][:, zb:zb + 1])
                            zs.append(z)
                            zq = hpool.tile([128, width], BF16, tag="zq")
                            nc.scalar.activation(zq[:], z[:], mybir.ActivationFunctionType.Square)
                            zqs.append(zq)
                        for zb in range(d.ZB):
                            nc.tensor.matmul(pn[:], ones_col[:], zqs[zb][:],
                                             start=(zb == 0), stop=(zb == d.ZB - 1))
                        # invn = exp(-0.5*ln(max(ssq,1e-24))) broadcast to 128 partitions
                        nm = npool.tile([1, width], F32, tag="nm")
                        nc.vector.tensor_scalar_max(nm[:], pn[:], 1e-24)
                        nc.scalar.activation(nm[:], nm[:], mybir.ActivationFunctionType.Ln)
                        nmb = npool.tile([1, width], BF16, tag="nmb")
                        nc.scalar.activation(nmb[:], nm[:], mybir.ActivationFunctionType.Exp, scale=-0.5)
                        pb = ppb.tile([128, width], F32, tag="pb")
                        nc.tensor.matmul(pb[:], ones_row[:], nmb[:], start=True, stop=True)
                        for zb in range(d.ZB):
                            dst = ZO[st][zb][:, mcl * 512:(mcl + 1) * 512] if own \
                                else ZT[st][zb][:, mc * 512:(mc + 1) * 512]
                            nc.vector.tensor_mul(dst, zs[zb][:], pb[:])

            # ================= Phase 4: logits + SupCon loss =================
            with (
                tc.tile_pool(name="msk", bufs=2) as mpool,
                tc.tile_pool(name="gsb", bufs=2) as gpool,
                tc.tile_pool(name="esb", bufs=1) as epool,
                tc.tile_pool(name="ldsb", bufs=2) as ldpool,
                tc.tile_pool(name="scr", bufs=1) as scrpool,
                tc.tile_pool(name="col", bufs=8) as colpool,
                tc.tile_pool(name="pg", bufs=4, space="PSUM") as ppg,
            ):
                for rb in range(d.RB if phases >= 4 else 0):
                    mp = mpool.tile([128, d.MP], BF16, tag="mp")
                    mn = mpool.tile([128, d.MP], BF16, tag="mn")
                    nc.sync.dma_start(mp[:], maskp[rb])
                    nc.sync.dma_start(mn[:], maskn[rb])
                    for dr in range(2):
                        an, co = (0, 1) if dr == 0 else (1, 0)
                        G = gpool.tile([128, d.MP], F32, tag="G")
                        for cc in range(d.MC):
                            pg = ppg.tile([128, 512], F32)
                            for zb in range(d.ZB):
                                nc.tensor.matmul(
                                    pg[:],
                                    ZO[an][zb][:, rb * 128:(rb + 1) * 128],
                                    ZT[co][zb][:, cc * 512:(cc + 1) * 512],
                                    start=(zb == 0), stop=(zb == d.ZB - 1))
                            nc.scalar.mul(G[:, cc * 512:(cc + 1) * 512], pg[:], 1.0 / TEMP)
                        mx = colpool.tile([128, 1], F32, tag="mx")
                        nc.vector.tensor_reduce(mx[:], G[:], axis=mybir.AxisListType.X, op=mybir.AluOpType.max)
                        negmx = colpool.tile([128, 1], F32, tag="negmx")
                        nc.vector.tensor_scalar_mul(negmx[:], mx[:], -1.0)
                        e = epool.tile([128, d.MP], F32, tag="e")
                        nc.scalar.activation(e[:], G[:], mybir.ActivationFunctionType.Exp, bias=negmx[:])
                        scr = scrpool.tile([128, d.MP], F32, tag="scr")
                        negr = colpool.tile([128, 1], F32, tag="negr")
                        nc.vector.tensor_mul(scr[:], e[:], mn[:])
                        nc.vector.tensor_reduce(negr[:], scr[:], axis=mybir.AxisListType.X, op=mybir.AluOpType.add)
                        nc.vector.tensor_scalar_add(e[:], e[:], negr[:])
                        ld = ldpool.tile([128, d.MP], F32, tag="ld")
                        nc.scalar.activation(ld[:], e[:], mybir.ActivationFunctionType.Ln)
                        # s = sum(maskpos * (l - ld)) ; l = G - mx
                        nc.vector.tensor_scalar_add(G[:], G[:], negmx[:])
                        nc.vector.tensor_sub(ld[:], G[:], ld[:])
                        nc.vector.tensor_mul(ld[:], ld[:], mp[:])
                        sd = colpool.tile([128, 1], F32, tag="sd")
                        nc.vector.tensor_reduce(sd[:], ld[:], axis=mybir.AxisListType.X, op=mybir.AluOpType.add)
                        nc.vector.tensor_mul(out_sb[:, dr * d.RB + rb:dr * d.RB + rb + 1], sd[:], rcsb[:, rb:rb + 1])
                if phases >= 4:
                    nc.sync.dma_start(pout[:], out_sb[:])
                elif phases == 3:
                    for st in range(2):
                        for zb in range(d.ZB):
                            nc.vector.tensor_copy(out_sb[:, (st*d.ZB+zb):(st*d.ZB+zb)+1], ZO[st][zb][:, 0:1])
                    nc.sync.dma_start(pout[:], out_sb[:])

    nc.compile()
    return nc


def prep_inputs(inputs, dims: Dims):
    """Host-side sharding/prep. Returns in_maps (one dict per core)."""
    d = dims
    f32 = np.float32
    preds = {0: inputs["preds_S"], 1: inputs["preds_T"]}
    sb = np.asarray(inputs["sample_batch"]).astype(np.int64)
    si = np.asarray(inputs["sample_idx"]).astype(np.int64)
    labels = np.asarray(inputs["labels_"])
    N = preds[0].shape[0]

    m_idx = np.arange(d.M)
    a_of_m = m_idx % d.A
    v_of_m = m_idx // d.A
    b_arr = sb[a_of_m]
    p_arr = si[a_of_m, v_of_m]

    W1 = {st: np.asarray(inputs[f"{p}_W1"]).astype(f32) for st, p in ((0, "s"), (1, "t"))}
    W2 = {st: np.asarray(inputs[f"{p}_W2"]).astype(f32) for st, p in ((0, "s"), (1, "t"))}

    # anchor pixels, channel-major, padded [2, KB, 128, MP]
    xst_np = np.zeros((2, d.C, d.MP), dtype=ml_dtypes.bfloat16)
    for st in range(2):
        X = np.asarray(preds[st]).reshape(N, d.C, d.HW)
        xs = X[b_arr, :, p_arr].astype(f32)  # [M, C]
        xst_np[st, :, :d.M] = xs.T.astype(ml_dtypes.bfloat16)
    xst_b = xst_np.reshape(2, d.KB, 128, d.MP)

    w1t_b = np.stack([W1[st].T.reshape(d.KB, 128, d.C) for st in range(2)]).astype(ml_dtypes.bfloat16)
    w2t_b = np.stack([W2[st].T.reshape(d.KB, 128, d.D) for st in range(2)]).astype(ml_dtypes.bfloat16)

    bnc_np = np.zeros((2, 3, 128, d.CB), f32)
    b2c_np = np.zeros((2, 128, d.ZB), f32)
    for st, p in ((0, "s"), (1, "t")):
        for j, nm in enumerate(("gamma", "beta", "b1")):
            bnc_np[st, j] = np.asarray(inputs[f"{p}_{nm}"]).astype(f32).reshape(d.CB, 128).T
        b2c_np[st] = np.asarray(inputs[f"{p}_b2"]).astype(f32).reshape(d.ZB, 128).T

    # masks
    base = (labels[:, None] == labels[None, :]).astype(f32)
    mask_full = np.tile(base, (d.V, d.V))
    mask_pos = mask_full * (1.0 - np.eye(d.M, dtype=f32))
    maskp_np = np.zeros((d.MP, d.MP), dtype=ml_dtypes.bfloat16)
    maskn_np = np.zeros((d.MP, d.MP), dtype=ml_dtypes.bfloat16)
    maskp_np[:d.M, :d.M] = mask_pos
    maskn_np[:d.M, :d.M] = (1.0 - mask_full)
    row_coef = np.zeros(d.MP, f32)
    row_coef[:d.M] = -LOSS_WEIGHT * (TEMP / BASE_TEMP) / d.M / (mask_pos.sum(axis=1) + 1e-6)

    in_maps = []
    for c in range(NCORES):
        m = {}
        img = np.stack([np.asarray(preds[st]).reshape(N, d.C, d.HW)[c % N] for st in range(2)])
        m["ximg"] = img.astype(ml_dtypes.bfloat16).reshape(2, d.KB, 128, d.HW)
        m["xst"] = xst_b
        r0, r1 = c * d.RPC, (c + 1) * d.RPC
        m["xso"] = np.ascontiguousarray(xst_b[:, :, :, r0:r1])
        m["w1t"] = w1t_b
        m["w2t"] = w2t_b
        m["bnc"] = bnc_np
        m["b2c"] = b2c_np
        m["maskp"] = np.ascontiguousarray(maskp_np[r0:r1].reshape(d.RB, 128, d.MP))
        m["maskn"] = np.ascontiguousarray(maskn_np[r0:r1].reshape(d.RB, 128, d.MP))
        m["rowco"] = np.ascontiguousarray(row_coef[r0:r1].reshape(d.RB, 128).T)
        in_maps.append(m)
    return in_maps


_CACHED = {}


def kernel(**inputs):
    dims = Dims()
    if "nc" not in _CACHED:
        _CACHED["nc"] = build_kernel(dims)
    nc = _CACHED["nc"]
    in_maps = prep_inputs(inputs, dims)
    res = bass_utils.run_bass_kernel_spmd(nc, in_maps, core_ids=list(range(NCORES)))
    total = np.float64(0.0)
    for r in res.results:
        total += np.float64(r["pout"].sum(dtype=np.float64))
    return np.float32(total)


if __name__ == "__main__":
    d = np.load("/root/problem/work/inputs.npz")
    inputs = {k: d[k] for k in d.files}
    expected = np.load("/root/problem/work/expected.npy")
    out = kernel(**inputs)
    print("expected:", expected, "actual:", out, "rel:", abs(out - expected) / abs(expected))

